# revision 5
# baseline (speedup 1.0000x reference)
"""CrossAttention TRN2 kernel (head-parallel, fp16 operands, host-summed partials).

Problem (hardcoded shapes):
  x    [4, 2048, 1024], cond [4, 2048, 1024]
  Wq/Wk/Wv [1024, 1024], Wo [1024, 1024], bo [1024]
  out = softmax((x@Wq) 8 heads of 128 @ (cond@Wk)^T * 0.125) @ (cond@Wv) @ Wo + bo

Sharding: 8 cores = (batch b in 0..3) x (head-half hh in 0..1).
Each core computes Q/K/V projections and attention for ITS 4 heads over the
full 2048 queries, then a PARTIAL output projection (contraction over its 512
inner columns only), written as fp16 [2048, 1024]. The host sums the two
partials per batch and adds the bias — no duplicated projection work and no
on-chip collectives. Total matmul rows/core: 532k.

Schedule notes:
  - fp16 matmul operands (1 cy/row), fp32 PSUM; PE sustains ~2.0 GHz.
  - xT/condT are DMA'd in 512-column chunks and the projection loops run
    ih/jh-outer so the first chains start after ~3MB instead of ~5MB.
  - Scores kept transposed [j, i]; softmax denominator accumulated on DVE in
    fp16, partition-reduced via ones-matmul, inverted with
    reciprocal_approx_fast.
  - Output projection is accumulated head-by-head into an fp16 SBUF
    accumulator (recycling xt's space) so it fills the exp-bound attention
    phase; per-(head,it) adds are merged to [128,1024] to cut DVE op count.
  - PSUM: tags sc 2x[128,1024] + av 2x[128,1024] = exactly 8 banks. The av
    accumulator is one [128,1024] tile per virtual head so consecutive heads
    overlap (the old 2x[128,512] layout serialized them).
"""
import numpy as np

import concourse.bacc as bacc
import concourse.tile as tile
from concourse import mybir
from concourse.bass_utils import run_bass_kernel_spmd

F32 = mybir.dt.float32
F16 = mybir.dt.float16
EXP = mybir.ActivationFunctionType.Exp

B, NQ, NK, D = 4, 2048, 2048, 1024
H, DH = 8, 128
SCALE = 64 ** -0.5
NCORES = 8
KT = D // 128                         # contraction tiles (8)
HL = 4                                # heads per core
JT = NK // 128                        # key tiles (16)
IT = NQ // 128                        # output row tiles (16)
MW = HL * DH                          # 512 inner columns per core


def build_nc():
    nc = bacc.Bacc()
    xT = nc.declare_dram_parameter("xT", [KT, 128, NQ], F16, isOutput=False)
    condT = nc.declare_dram_parameter("condT", [KT, 128, NK], F16, isOutput=False)
    wq = nc.declare_dram_parameter("wq", [KT, 128, MW], F16, isOutput=False)
    wk = nc.declare_dram_parameter("wk", [KT, 128, MW], F16, isOutput=False)
    wv = nc.declare_dram_parameter("wv", [KT, 128, MW], F16, isOutput=False)
    wo = nc.declare_dram_parameter("wo", [HL, 128, D], F16, isOutput=False)
    out = nc.declare_dram_parameter("out", [NQ, D], F16, isOutput=True)

    with tile.TileContext(nc) as tc:
        with (
            nc.allow_low_precision(reason="fp16 matmul operands are intended"),
            tc.tile_pool(name="const", bufs=1) as const,
            tc.tile_pool(name="big", bufs=1) as big,
            tc.tile_pool(name="expp", bufs=4) as expp,
            tc.tile_pool(name="den", bufs=2) as denp,
            tc.tile_pool(name="ps", bufs=1, space="PSUM") as ps,
        ):
            ones = const.tile([128, 128], F16)
            nc.vector.memset(ones, 1.0)

            wq_s = big.tile([128, KT, MW], F16, tag="wq_s")
            xt = big.tile([128, KT, NQ], F16, tag="xt")
            wk_s = big.tile([128, KT, MW], F16, tag="wk_s")
            ct = big.tile([128, KT, NK], F16, tag="ct")
            wv_s = big.tile([128, KT, MW], F16, tag="wv_s")
            wo_s = big.tile([128, HL, D], F16, tag="wo_s")
            qT = big.tile([128, HL, NQ], F16, tag="qT")
            kT = big.tile([128, HL, NK], F16, tag="kT")
            v = big.tile([128, JT, MW], F16, tag="v")
            attT = big.tile([128, HL, NQ], F16, tag="attT")

            # DMA issue order = arrival priority. 512-col chunks so the
            # ih/jh-outer projection loops start early.
            for k in range(KT):
                nc.sync.dma_start(out=wq_s[:, k, :], in_=wq[k, :, :])
                nc.sync.dma_start(out=xt[:, k, 0:512], in_=xT[k, :, 0:512])
            for q in range(1, 4):
                cs = slice(q * 512, (q + 1) * 512)
                for k in range(KT):
                    nc.sync.dma_start(out=xt[:, k, cs], in_=xT[k, :, cs])
            for k in range(KT):
                nc.sync.dma_start(out=wk_s[:, k, :], in_=wk[k, :, :])
            for q in range(4):
                cs = slice(q * 512, (q + 1) * 512)
                for k in range(KT):
                    nc.sync.dma_start(out=ct[:, k, cs], in_=condT[k, :, cs])
            for k in range(KT):
                nc.sync.dma_start(out=wv_s[:, k, :], in_=wv[k, :, :])
            for h in range(HL):
                nc.sync.dma_start(out=wo_s[:, h, :], in_=wo[h, :, :])

            # ---- Q projection: qT[:, h, :] = Wq_h.T @ xT ----
            for ih in range(NQ // 512):
                for h in range(HL):
                    acc = ps.tile([128, 512], F32, tag="sc", bufs=2,
                                  name=f"accq_{h}_{ih}")
                    for k in range(KT):
                        nc.tensor.matmul(
                            acc,
                            wq_s[:, k, h * DH:(h + 1) * DH],
                            xt[:, k, ih * 512:(ih + 1) * 512],
                            start=(k == 0), stop=(k == KT - 1))
                    nc.vector.tensor_copy(qT[:, h, ih * 512:(ih + 1) * 512], acc)

            # ---- K projection: kT[:, h, :] = Wk_h.T @ condT ----
            for jh in range(NK // 512):
                for h in range(HL):
                    acc = ps.tile([128, 512], F32, tag="sc", bufs=2,
                                  name=f"acck_{h}_{jh}")
                    for k in range(KT):
                        nc.tensor.matmul(
                            acc,
                            wk_s[:, k, h * DH:(h + 1) * DH],
                            ct[:, k, jh * 512:(jh + 1) * 512],
                            start=(k == 0), stop=(k == KT - 1))
                    nc.vector.tensor_copy(kT[:, h, jh * 512:(jh + 1) * 512], acc)

            # ---- V projection: v[:, jt, :] = condT_jt.T @ Wv (4 heads wide) ----
            for jt in range(JT):
                acc = ps.tile([128, MW], F32, tag="sc", bufs=2,
                              name=f"accv_{jt}")
                for k in range(KT):
                    nc.tensor.matmul(
                        acc,
                        ct[:, k, jt * 128:(jt + 1) * 128],
                        wv_s[:, k, :],
                        start=(k == 0), stop=(k == KT - 1))
                nc.vector.tensor_copy(v[:, jt, :], acc)

            # output accumulator recycles xt's SBUF (xt is dead after Q proj)
            out_acc = big.tile([128, IT, D], F16, tag="xt")

            # ---- attention per head (two 1024-query halves each) ----
            for h in range(HL):
                for half in range(2):
                    i0 = half * 1024
                    av = ps.tile([128, 1024], F32, tag="av", bufs=2,
                                 name=f"av_{h}_{half}")
                    den_s = denp.tile([128, 1024], F16, tag="den_s",
                                      name=f"den_s_{h}_{half}")
                    for jt in range(JT):
                        sc = ps.tile([128, 1024], F32, tag="sc", bufs=2,
                                     name=f"sc_{h}_{half}_{jt}")
                        for ih in range(2):
                            nc.tensor.matmul(
                                sc[:, ih * 512:(ih + 1) * 512],
                                kT[:, h, jt * 128:(jt + 1) * 128],
                                qT[:, h, i0 + ih * 512:i0 + (ih + 1) * 512],
                                start=True, stop=True)
                        esc = expp.tile([128, 1024], F16, tag="esc",
                                        name=f"esc_{h}_{half}_{jt}")
                        nc.scalar.activation(esc, sc, EXP)
                        for ih in range(2):
                            nc.tensor.matmul(
                                av[:, ih * 512:(ih + 1) * 512],
                                v[:, jt, h * DH:(h + 1) * DH],
                                esc[:, ih * 512:(ih + 1) * 512],
                                start=(jt == 0), stop=(jt == JT - 1))
                        with tc.high_priority():
                            if jt == 0:
                                nc.vector.tensor_copy(den_s, esc)
                            else:
                                nc.vector.tensor_add(den_s, den_s, esc)
                    den_bc = ps.tile([128, 1024], F32, tag="sc", bufs=2,
                                     name=f"den_bc_{h}_{half}")
                    den_rec = denp.tile([128, 1024], F32, tag="den_rec",
                                        name=f"den_rec_{h}_{half}")
                    with tc.high_priority():
                        for ih in range(2):
                            nc.tensor.matmul(
                                den_bc[:, ih * 512:(ih + 1) * 512],
                                ones,
                                den_s[:, ih * 512:(ih + 1) * 512],
                                start=True, stop=True)
                        nc.vector.reciprocal_approx_fast(out=den_rec, in_=den_bc)
                        nc.vector.tensor_mul(
                            attT[:, h, i0:i0 + 1024], av, den_rec)

                # ---- partial O projection contribution of head h ----
                # out_acc[:, it, :] (+)= attT_h[:, it-tile].T @ Wo_h
                for it in range(IT):
                    fo = ps.tile([128, D], F32, tag="sc", bufs=2,
                                 name=f"fo_{h}_{it}")
                    for nh in range(2):
                        nc.tensor.matmul(
                            fo[:, nh * 512:(nh + 1) * 512],
                            attT[:, h, it * 128:(it + 1) * 128],
                            wo_s[:, h, nh * 512:(nh + 1) * 512],
                            start=True, stop=True)
                    dst = out_acc[:, it, :]
                    if h == 0:
                        nc.vector.tensor_copy(dst, fo)
                    else:
                        nc.vector.tensor_add(dst, dst, fo)
                    if h == HL - 1:
                        nc.sync.dma_start(
                            out=out[it * 128:(it + 1) * 128, :], in_=dst)
    nc.finalize()
    return nc


_NC_CACHE = None


def _get_nc():
    global _NC_CACHE
    if _NC_CACHE is None:
        _NC_CACHE = build_nc()
    return _NC_CACHE


def make_in_maps(x, cond, Wq, Wk, Wv, Wo, bo):
    wq16 = (np.asarray(Wq, np.float32) * SCALE).astype(np.float16)
    wk16 = np.asarray(Wk, np.float32).astype(np.float16)
    wv16 = np.asarray(Wv, np.float32).astype(np.float16)
    wo16 = np.asarray(Wo, np.float32).astype(np.float16)
    x16 = np.asarray(x, np.float32).astype(np.float16)
    c16 = np.asarray(cond, np.float32).astype(np.float16)
    in_maps = []
    for c in range(NCORES):
        b, hh = c // 2, c % 2
        cols = slice(hh * MW, (hh + 1) * MW)
        in_maps.append({
            "xT": np.ascontiguousarray(x16[b].T).reshape(KT, 128, NQ),
            "condT": np.ascontiguousarray(c16[b].T).reshape(KT, 128, NK),
            "wq": np.ascontiguousarray(wq16[:, cols]).reshape(KT, 128, MW),
            "wk": np.ascontiguousarray(wk16[:, cols]).reshape(KT, 128, MW),
            "wv": np.ascontiguousarray(wv16[:, cols]).reshape(KT, 128, MW),
            "wo": np.ascontiguousarray(wo16[cols, :]).reshape(HL, 128, D),
        })
    return in_maps


def kernel(x, cond, Wq, Wk, Wv, Wo, bo, _trace=False, _trace_kwargs=None):
    nc = _get_nc()
    in_maps = make_in_maps(x, cond, Wq, Wk, Wv, Wo, bo)
    kw = {}
    if _trace:
        kw = {"trace": True, "trace_kwargs": _trace_kwargs or {}}
    res = run_bass_kernel_spmd(nc, in_maps, list(range(NCORES)), **kw)
    bo32 = np.asarray(bo, np.float32)
    out = np.empty((B, NQ, D), dtype=np.float32)
    for b in range(B):
        out[b] = (res.results[2 * b]["out"].astype(np.float32)
                  + res.results[2 * b + 1]["out"].astype(np.float32) + bo32)
    if _trace:
        return out, res
    return out


if __name__ == "__main__":
    rng = np.random.default_rng(0)
    s = 0.02
    x = rng.standard_normal((B, NQ, D), dtype=np.float32)
    cond = rng.standard_normal((B, NK, D), dtype=np.float32)
    Wq = (rng.standard_normal((D, D), dtype=np.float32) * s)
    Wk = (rng.standard_normal((D, D), dtype=np.float32) * s)
    Wv = (rng.standard_normal((D, D), dtype=np.float32) * s)
    Wo = (rng.standard_normal((D, D), dtype=np.float32) * s)
    bo = (rng.standard_normal((D,), dtype=np.float32) * s)

    def ref_np(x, cond):
        q = (x @ Wq).reshape(B, NQ, H, DH).transpose(0, 2, 1, 3)
        k = (cond @ Wk).reshape(B, NK, H, DH).transpose(0, 2, 1, 3)
        v = (cond @ Wv).reshape(B, NK, H, DH).transpose(0, 2, 1, 3)
        sim = np.einsum('bhid,bhjd->bhij', q, k) * SCALE
        sim = sim - sim.max(axis=-1, keepdims=True)
        a = np.exp(sim)
        a = a / a.sum(axis=-1, keepdims=True)
        o = np.einsum('bhij,bhjd->bhid', a, v)
        o = o.transpose(0, 2, 1, 3).reshape(B, NQ, D)
        return o @ Wo + bo

    import time
    t0 = time.time()
    got = kernel(x=x, cond=cond, Wq=Wq, Wk=Wk, Wv=Wv, Wo=Wo, bo=bo)
    print(f"kernel run {time.time()-t0:.1f}s")
    exp = ref_np(x.astype(np.float64), cond.astype(np.float64))
    err = np.abs(got - exp)
    rel = np.linalg.norm(got - exp) / np.linalg.norm(exp)
    print(f"rel_l2={rel:.3e} absmax_rel={err.max()/np.abs(exp).max():.3e}")


# revision 6
# speedup vs baseline: 1.2822x; 1.2822x over previous
"""CrossAttention TRN2 kernel (head-parallel, fp16 operands, host-summed partials).

Problem (hardcoded shapes):
  x    [4, 2048, 1024], cond [4, 2048, 1024]
  Wq/Wk/Wv [1024, 1024], Wo [1024, 1024], bo [1024]
  out = softmax((x@Wq) 8 heads of 128 @ (cond@Wk)^T * 0.125) @ (cond@Wv) @ Wo + bo

Sharding: 8 cores = (batch b in 0..3) x (head-half hh in 0..1).
Each core computes Q/K/V projections and attention for ITS 4 heads over the
full 2048 queries, then a PARTIAL output projection (contraction over its 512
inner columns only), written as fp16 [2048, 1024]. The host sums the two
partials per batch and adds the bias — no duplicated projection work and no
on-chip collectives. Total matmul rows/core: 532k.

Schedule notes:
  - fp16 matmul operands (1 cy/row), fp32 PSUM; PE sustains ~2.0 GHz.
  - xT/condT are DMA'd in 512-col chunks with ih/jh-outer projection loops so
    the first chains start after ~3MB instead of ~5MB.
  - Scores kept transposed [j, i]; softmax denominator accumulated on DVE in
    fp16, partition-reduced via ones-matmul, inverted with
    reciprocal_approx_fast.
  - Output projection is accumulated head-by-head into an fp16 SBUF
    accumulator (recycling xt's space); its PSUM staging tiles and the
    projection accumulators share the "av" tag so the exp-critical score
    pipeline ("sc" tag) is never paced by DVE adds. expp is 8 deep so av
    matmuls may lag while previous head's O-adds drain.
  - PSUM: sc 2x[128,1024] + av-tag 2x[128,1024] = exactly 8 banks.
"""
import numpy as np

import concourse.bacc as bacc
import concourse.tile as tile
from concourse import mybir
from concourse.bass_utils import run_bass_kernel_spmd

F32 = mybir.dt.float32
F16 = mybir.dt.float16
EXP = mybir.ActivationFunctionType.Exp

B, NQ, NK, D = 4, 2048, 2048, 1024
H, DH = 8, 128
SCALE = 64 ** -0.5
NCORES = 8
KT = D // 128                         # contraction tiles (8)
HL = 4                                # heads per core
JT = NK // 128                        # key tiles (16)
IT = NQ // 128                        # output row tiles (16)
MW = HL * DH                          # 512 inner columns per core


def build_nc():
    nc = bacc.Bacc()
    xT = nc.declare_dram_parameter("xT", [KT, 128, NQ], F16, isOutput=False)
    condT = nc.declare_dram_parameter("condT", [KT, 128, NK], F16, isOutput=False)
    wq = nc.declare_dram_parameter("wq", [KT, 128, MW], F16, isOutput=False)
    wk = nc.declare_dram_parameter("wk", [KT, 128, MW], F16, isOutput=False)
    wv = nc.declare_dram_parameter("wv", [KT, 128, MW], F16, isOutput=False)
    wo = nc.declare_dram_parameter("wo", [HL, 128, D], F16, isOutput=False)
    out = nc.declare_dram_parameter("out", [NQ, D], F16, isOutput=True)

    with tile.TileContext(nc) as tc:
        with (
            nc.allow_low_precision(reason="fp16 matmul operands are intended"),
            tc.tile_pool(name="const", bufs=1) as const,
            tc.tile_pool(name="big", bufs=1) as big,
            tc.tile_pool(name="expp", bufs=8) as expp,
            tc.tile_pool(name="den", bufs=2) as denp,
            tc.tile_pool(name="ps", bufs=1, space="PSUM") as ps,
        ):
            ones = const.tile([128, 128], F16)
            nc.vector.memset(ones, 1.0)

            wq_s = big.tile([128, KT, MW], F16, tag="wq_s")
            xt = big.tile([128, KT, NQ], F16, tag="xt")
            wk_s = big.tile([128, KT, MW], F16, tag="wk_s")
            ct = big.tile([128, KT, NK], F16, tag="ct")
            wv_s = big.tile([128, KT, MW], F16, tag="wv_s")
            wo_s = big.tile([128, HL, D], F16, tag="wo_s")
            qT = big.tile([128, HL, NQ], F16, tag="qT")
            kT = big.tile([128, HL, NK], F16, tag="kT")
            v = big.tile([128, JT, MW], F16, tag="v")
            attT = big.tile([128, HL, NQ], F16, tag="attT")

            # DMA issue order = arrival priority (512-col chunks).
            for k in range(KT):
                nc.sync.dma_start(out=wq_s[:, k, :], in_=wq[k, :, :])
                nc.sync.dma_start(out=xt[:, k, 0:512], in_=xT[k, :, 0:512])
            for q in range(1, 4):
                cs = slice(q * 512, (q + 1) * 512)
                for k in range(KT):
                    nc.sync.dma_start(out=xt[:, k, cs], in_=xT[k, :, cs])
            for k in range(KT):
                nc.sync.dma_start(out=wk_s[:, k, :], in_=wk[k, :, :])
            for q in range(4):
                cs = slice(q * 512, (q + 1) * 512)
                for k in range(KT):
                    nc.sync.dma_start(out=ct[:, k, cs], in_=condT[k, :, cs])
            for k in range(KT):
                nc.sync.dma_start(out=wv_s[:, k, :], in_=wv[k, :, :])
            for h in range(HL):
                nc.sync.dma_start(out=wo_s[:, h, :], in_=wo[h, :, :])

            # ---- Q projection: qT[:, h, :] = Wq_h.T @ xT ----
            for ih in range(NQ // 512):
                for h in range(HL):
                    acc = ps.tile([128, 512], F32, tag="av", bufs=2,
                                  name=f"accq_{h}_{ih}")
                    for k in range(KT):
                        nc.tensor.matmul(
                            acc,
                            wq_s[:, k, h * DH:(h + 1) * DH],
                            xt[:, k, ih * 512:(ih + 1) * 512],
                            start=(k == 0), stop=(k == KT - 1))
                    nc.vector.tensor_copy(qT[:, h, ih * 512:(ih + 1) * 512], acc)

            # ---- K projection: kT[:, h, :] = Wk_h.T @ condT ----
            for jh in range(NK // 512):
                for h in range(HL):
                    acc = ps.tile([128, 512], F32, tag="av", bufs=2,
                                  name=f"acck_{h}_{jh}")
                    for k in range(KT):
                        nc.tensor.matmul(
                            acc,
                            wk_s[:, k, h * DH:(h + 1) * DH],
                            ct[:, k, jh * 512:(jh + 1) * 512],
                            start=(k == 0), stop=(k == KT - 1))
                    nc.vector.tensor_copy(kT[:, h, jh * 512:(jh + 1) * 512], acc)

            # ---- V projection: v[:, jt, :] = condT_jt.T @ Wv (4 heads wide) ----
            for jt in range(JT):
                acc = ps.tile([128, MW], F32, tag="av", bufs=2,
                              name=f"accv_{jt}")
                for k in range(KT):
                    nc.tensor.matmul(
                        acc,
                        ct[:, k, jt * 128:(jt + 1) * 128],
                        wv_s[:, k, :],
                        start=(k == 0), stop=(k == KT - 1))
                nc.vector.tensor_copy(v[:, jt, :], acc)

            # output accumulator recycles xt's SBUF (xt is dead after Q proj)
            out_acc = big.tile([128, IT, D], F16, tag="xt")

            # ---- attention per head (two 1024-query halves each) ----
            for h in range(HL):
                for half in range(2):
                    i0 = half * 1024
                    av = ps.tile([128, 1024], F32, tag="av", bufs=2,
                                 name=f"av_{h}_{half}")
                    den_s = denp.tile([128, 1024], F16, tag="den_s",
                                      name=f"den_s_{h}_{half}")
                    for jt in range(JT):
                        sc = ps.tile([128, 1024], F32, tag="sc", bufs=2,
                                     name=f"sc_{h}_{half}_{jt}")
                        for ih in range(2):
                            nc.tensor.matmul(
                                sc[:, ih * 512:(ih + 1) * 512],
                                kT[:, h, jt * 128:(jt + 1) * 128],
                                qT[:, h, i0 + ih * 512:i0 + (ih + 1) * 512],
                                start=True, stop=True)
                        esc = expp.tile([128, 1024], F16, tag="esc",
                                        name=f"esc_{h}_{half}_{jt}")
                        nc.scalar.activation(esc, sc, EXP)
                        for ih in range(2):
                            nc.tensor.matmul(
                                av[:, ih * 512:(ih + 1) * 512],
                                v[:, jt, h * DH:(h + 1) * DH],
                                esc[:, ih * 512:(ih + 1) * 512],
                                start=(jt == 0), stop=(jt == JT - 1))
                        with tc.high_priority():
                            if jt == 0:
                                nc.vector.tensor_copy(den_s, esc)
                            else:
                                nc.vector.tensor_add(den_s, den_s, esc)
                    den_bc = ps.tile([128, 1024], F32, tag="sc", bufs=2,
                                     name=f"den_bc_{h}_{half}")
                    den_rec = denp.tile([128, 1024], F32, tag="den_rec",
                                        name=f"den_rec_{h}_{half}")
                    with tc.high_priority():
                        for ih in range(2):
                            nc.tensor.matmul(
                                den_bc[:, ih * 512:(ih + 1) * 512],
                                ones,
                                den_s[:, ih * 512:(ih + 1) * 512],
                                start=True, stop=True)
                        nc.vector.reciprocal_approx_fast(out=den_rec, in_=den_bc)
                        nc.vector.tensor_mul(
                            attT[:, h, i0:i0 + 1024], av, den_rec)

                # ---- partial O projection contribution of head h ----
                # out_acc[:, it, :] (+)= attT_h[:, it-tile].T @ Wo_h
                for it in range(IT):
                    fo = ps.tile([128, D], F32, tag="av", bufs=2,
                                 name=f"fo_{h}_{it}")
                    for nh in range(2):
                        nc.tensor.matmul(
                            fo[:, nh * 512:(nh + 1) * 512],
                            attT[:, h, it * 128:(it + 1) * 128],
                            wo_s[:, h, nh * 512:(nh + 1) * 512],
                            start=True, stop=True)
                    dst = out_acc[:, it, :]
                    if h == 0:
                        nc.vector.tensor_copy(dst, fo)
                    else:
                        nc.vector.tensor_add(dst, dst, fo)
                    if h == HL - 1:
                        nc.sync.dma_start(
                            out=out[it * 128:(it + 1) * 128, :], in_=dst)
    nc.finalize()
    return nc


_NC_CACHE = None


def _get_nc():
    global _NC_CACHE
    if _NC_CACHE is None:
        _NC_CACHE = build_nc()
    return _NC_CACHE


def make_in_maps(x, cond, Wq, Wk, Wv, Wo, bo):
    wq16 = (np.asarray(Wq, np.float32) * SCALE).astype(np.float16)
    wk16 = np.asarray(Wk, np.float32).astype(np.float16)
    wv16 = np.asarray(Wv, np.float32).astype(np.float16)
    wo16 = np.asarray(Wo, np.float32).astype(np.float16)
    x16 = np.asarray(x, np.float32).astype(np.float16)
    c16 = np.asarray(cond, np.float32).astype(np.float16)
    in_maps = []
    for c in range(NCORES):
        b, hh = c // 2, c % 2
        cols = slice(hh * MW, (hh + 1) * MW)
        in_maps.append({
            "xT": np.ascontiguousarray(x16[b].T).reshape(KT, 128, NQ),
            "condT": np.ascontiguousarray(c16[b].T).reshape(KT, 128, NK),
            "wq": np.ascontiguousarray(wq16[:, cols]).reshape(KT, 128, MW),
            "wk": np.ascontiguousarray(wk16[:, cols]).reshape(KT, 128, MW),
            "wv": np.ascontiguousarray(wv16[:, cols]).reshape(KT, 128, MW),
            "wo": np.ascontiguousarray(wo16[cols, :]).reshape(HL, 128, D),
        })
    return in_maps


def kernel(x, cond, Wq, Wk, Wv, Wo, bo, _trace=False, _trace_kwargs=None):
    nc = _get_nc()
    in_maps = make_in_maps(x, cond, Wq, Wk, Wv, Wo, bo)
    kw = {}
    if _trace:
        kw = {"trace": True, "trace_kwargs": _trace_kwargs or {}}
    res = run_bass_kernel_spmd(nc, in_maps, list(range(NCORES)), **kw)
    bo32 = np.asarray(bo, np.float32)
    out = np.empty((B, NQ, D), dtype=np.float32)
    for b in range(B):
        out[b] = (res.results[2 * b]["out"].astype(np.float32)
                  + res.results[2 * b + 1]["out"].astype(np.float32) + bo32)
    if _trace:
        return out, res
    return out


if __name__ == "__main__":
    rng = np.random.default_rng(0)
    s = 0.02
    x = rng.standard_normal((B, NQ, D), dtype=np.float32)
    cond = rng.standard_normal((B, NK, D), dtype=np.float32)
    Wq = (rng.standard_normal((D, D), dtype=np.float32) * s)
    Wk = (rng.standard_normal((D, D), dtype=np.float32) * s)
    Wv = (rng.standard_normal((D, D), dtype=np.float32) * s)
    Wo = (rng.standard_normal((D, D), dtype=np.float32) * s)
    bo = (rng.standard_normal((D,), dtype=np.float32) * s)

    def ref_np(x, cond):
        q = (x @ Wq).reshape(B, NQ, H, DH).transpose(0, 2, 1, 3)
        k = (cond @ Wk).reshape(B, NK, H, DH).transpose(0, 2, 1, 3)
        v = (cond @ Wv).reshape(B, NK, H, DH).transpose(0, 2, 1, 3)
        sim = np.einsum('bhid,bhjd->bhij', q, k) * SCALE
        sim = sim - sim.max(axis=-1, keepdims=True)
        a = np.exp(sim)
        a = a / a.sum(axis=-1, keepdims=True)
        o = np.einsum('bhij,bhjd->bhid', a, v)
        o = o.transpose(0, 2, 1, 3).reshape(B, NQ, D)
        return o @ Wo + bo

    import time
    t0 = time.time()
    got = kernel(x=x, cond=cond, Wq=Wq, Wk=Wk, Wv=Wv, Wo=Wo, bo=bo)
    print(f"kernel run {time.time()-t0:.1f}s")
    exp = ref_np(x.astype(np.float64), cond.astype(np.float64))
    err = np.abs(got - exp)
    rel = np.linalg.norm(got - exp) / np.linalg.norm(exp)
    print(f"rel_l2={rel:.3e} absmax_rel={err.max()/np.abs(exp).max():.3e}")
